# revision 1
# baseline (speedup 1.0000x reference)
"""Trainium2 Bass kernel for nn_DiffusionStar (retrieval_knn).

Computes eps_star = (x - sqrt(ab) * weighted_x) / sqrt(1 - ab) where
weighted_x is the softmax-weighted average of the train set under the
Gaussian kernel exp(-||x - sqrt(ab) x0||^2 / (2 (1 - ab))).

Strategy: shard train_data along N across 8 cores; each core streams its
shard once with online softmax (running max m, exp-sum s, weighted sum W,
argmax index). All matmuls run in fp8(e4m3) DoubleRow perf mode (2
contraction rows/cycle):
 - logits: d-major train copy split into e4m3 hi + e4m3 lo(x16) operands;
   x is packed the same way, so the cross product keeps ~fp16 accuracy
   (validated: top-2 logit gaps ~4, fp8-hi/lo logit error ~0.3).
 - W = p @ train: n-major e4m3 copy. The softmax is effectively 1-NN
   (top-1 weight > 0.97) and the argmax row enters W with coefficient
   exactly exp(0) = 1.0, so the host swaps that row's fp8 value for the
   exact f32 row (per-core repair). Cores return (W, m, s, idx); host
   merges with the online-softmax combine.
"""

import contextlib

import ml_dtypes
import numpy as np

from concourse import bacc, bass, mybir, tile
from concourse import bass_utils

FP16 = mybir.dt.float16
FP8 = mybir.dt.float8e4
F32 = mybir.dt.float32
NP_FP8 = ml_dtypes.float8_e4m3

B = 32          # queries
D = 3072        # feature dim (c*h*w)
N = 100000      # train points
N_CORES = 8
N_SHARD = N // N_CORES          # 12500
TILE = 512
N_TILES = (N_SHARD + TILE - 1) // TILE   # 25
N_PAD = N_TILES * TILE                   # 12800
KC = D // 128                            # 24 contraction chunks
KP = KC // 2                             # 12 DoubleRow chunk pairs
LO_SCALE = 16.0                          # fp8 lo-residual scale
PAD_BIAS = -30000.0                      # logit bias for padded rows
DR = mybir.MatmulPerfMode.DoubleRow


def build_nc(n_tiles=N_TILES, repeat=1, skip_compute=False, skip_dma=False,
             qmode="two_bal"):
    nc = bacc.Bacc("TRN2", target_bir_lowering=False, debug=False, num_devices=1)

    # d-major fp8 hi/lo pair operands: [tile, p, pair, 2, n] so each SBUF
    # partition's bytes are one contiguous HBM run — max DMA efficiency
    a8h = nc.dram_tensor(
        "a8h", [n_tiles, 128, KP, 2, TILE], FP8, kind="ExternalInput"
    ).ap()
    a8l = nc.dram_tensor(
        "a8l", [n_tiles, 128, KP, 2, TILE], FP8, kind="ExternalInput"
    ).ap()
    a_b = nc.dram_tensor("a_b", [2, n_tiles, TILE], FP16, kind="ExternalInput").ap()
    b8 = nc.dram_tensor("b8", [n_tiles, 128, 4, D], FP8, kind="ExternalInput").ap()
    # xw8[0] = (x8h | x8l); xw8[1] = (x8h/16 | x8l/256) for the lo pass,
    # pre-scaled so both passes accumulate into one PSUM tile
    xw8 = nc.dram_tensor("xw8", [2, KP, 128, 2, 64], FP8, kind="ExternalInput").ap()
    xwb = nc.dram_tensor("xwb", [2, 64], FP16, kind="ExternalInput").ap()
    ident = nc.dram_tensor("ident", [32, 32], F32, kind="ExternalInput").ap()
    iota = nc.dram_tensor("iota", [32, TILE], F32, kind="ExternalInput").ap()

    w_out = nc.dram_tensor("w_out", [B, D], F32, kind="ExternalOutput").ap()
    m_out = nc.dram_tensor("m_out", [B, 1], F32, kind="ExternalOutput").ap()
    s_out = nc.dram_tensor("s_out", [B, 1], F32, kind="ExternalOutput").ap()
    i_out = nc.dram_tensor("i_out", [B, 1], F32, kind="ExternalOutput").ap()

    with tile.TileContext(nc) as tc, contextlib.ExitStack() as st:
        const = st.enter_context(tc.tile_pool(name="const", bufs=1))
        apool = st.enter_context(tc.tile_pool(name="apool", bufs=3))
        bpool = st.enter_context(tc.tile_pool(name="bpool", bufs=3))
        small = st.enter_context(tc.tile_pool(name="small", bufs=3))
        pwork = st.enter_context(tc.tile_pool(name="pwork", bufs=2))
        ps_c1 = st.enter_context(tc.tile_pool(name="ps_c1", bufs=1, space="PSUM"))
        ps_pt = st.enter_context(tc.tile_pool(name="ps_pt", bufs=1, space="PSUM"))
        ps_w = st.enter_context(tc.tile_pool(name="ps_w", bufs=1, space="PSUM"))

        xw8_s = const.tile([128, 2, KP, 2, 64], FP8)
        nc.sync.dma_start(xw8_s[:], xw8.rearrange("s k p r j -> p s k r j"))
        xwb_s = const.tile([2, 64], FP16)
        nc.sync.dma_start(xwb_s[:], xwb)
        ident_s = const.tile([32, 32], F32)
        nc.sync.dma_start(ident_s[:], ident)
        iota_s = const.tile([32, TILE], F32)
        nc.sync.dma_start(iota_s[:], iota)

        W_acc = const.tile([B, D], F32)
        nc.vector.memset(W_acc[:], 0.0)
        m_run = const.tile([B, 1], F32)
        nc.vector.memset(m_run[:], -1e30)
        s_run = const.tile([B, 1], F32)
        nc.vector.memset(s_run[:], 0.0)
        i_run = const.tile([B, 1], F32)
        nc.vector.memset(i_run[:], 0.0)

        def emit_tile(i):
            a8h_t = apool.tile([128, KP, 2, TILE], FP8, tag="a8h")
            a8l_t = apool.tile([128, KP, 2, TILE], FP8, tag="a8l")
            b8_t = bpool.tile([128, 4, D], FP8, tag="b8")
            ab_t = apool.tile([2, TILE], FP16, tag="ab")
            if not skip_dma:
                # TRN2 has exactly 2 HW DGE queues: SP (sync) and Act (scalar)
                if qmode == "two_bal":
                    nc.sync.dma_start(a8h_t[:], a8h[i])
                    nc.sync.dma_start(b8_t[:, :, 0 : D // 2], b8[i, :, :, 0 : D // 2])
                    nc.scalar.dma_start(a8l_t[:], a8l[i])
                    nc.scalar.dma_start(b8_t[:, :, D // 2 : D], b8[i, :, :, D // 2 : D])
                    nc.sync.dma_start(ab_t[:], a_b[:, i])
                elif qmode == "two_raw":
                    nc.sync.dma_start(a8h_t[:], a8h[i])
                    nc.scalar.dma_start(a8l_t[:], a8l[i])
                    nc.scalar.dma_start(b8_t[:], b8[i])
                    nc.sync.dma_start(ab_t[:], a_b[:, i])
                elif qmode == "one":
                    nc.sync.dma_start(a8h_t[:], a8h[i])
                    nc.sync.dma_start(a8l_t[:], a8l[i])
                    nc.sync.dma_start(b8_t[:], b8[i])
                    nc.sync.dma_start(ab_t[:], a_b[:, i])
                else:  # "swdge3": 2 HW queues + Pool software DGE
                    nc.sync.dma_start(a8h_t[:], a8h[i])
                    nc.gpsimd.dma_start(a8l_t[:], a8l[i])
                    nc.sync.dma_start(ab_t[:], a_b[:, i])
                    nc.scalar.dma_start(b8_t[:], b8[i])
            if skip_dma:
                nc.vector.memset(a8h_t[:, 0, 0, 0:2], 0.0)
                nc.vector.memset(a8l_t[:, 0, 0, 0:2], 0.0)
                nc.vector.memset(ab_t[:, 0:2], 0.0)
                nc.vector.memset(b8_t[:, 0, 0:2], 0.0)
            if skip_compute:
                dmy = small.tile([128, 1], F32, tag="dmy")
                nc.vector.reduce_max(dmy[:], a8h_t[:, 0, 0, 0:8], axis=mybir.AxisListType.X)
                nc.vector.reduce_max(dmy[:], a8l_t[:, 0, 0, 0:8], axis=mybir.AxisListType.X)
                nc.vector.reduce_max(dmy[:], b8_t[:, 0, 0:8], axis=mybir.AxisListType.X)
                return

            # hi pass with (x8h|x8l), lo pass with (x8h/16|x8l/256), plus bias;
            # all accumulate into one PSUM tile
            c1 = ps_c1.tile([64, TILE], F32, tag="c1")
            for j in range(KP):
                nc.tensor.matmul(
                    c1[:], xw8_s[:, 0, j], a8h_t[:, j],
                    start=(j == 0), stop=False, perf_mode=DR,
                )
            for j in range(KP):
                nc.tensor.matmul(
                    c1[:], xw8_s[:, 1, j], a8l_t[:, j],
                    start=False, stop=False, perf_mode=DR,
                )
            nc.tensor.matmul(c1[:], xwb_s[:], ab_t[:], start=False, stop=True)

            # logits = rows_h + rows_l/16  (bias already in rows_h);
            # only one PSUM operand allowed per op: copy hi rows to SBUF first
            crossb = pwork.tile([B, TILE], F32, tag="crossb")
            nc.scalar.copy(crossb[:], c1[0:B, :])
            logits = pwork.tile([B, TILE], F32, tag="logits")
            nc.vector.scalar_tensor_tensor(
                logits[:], c1[B:64, :], 1.0 / LO_SCALE, crossb[:],
                mybir.AluOpType.mult, mybir.AluOpType.add,
            )

            mt = small.tile([B, 1], F32, tag="mt")
            nc.vector.reduce_max(mt[:], logits[:], axis=mybir.AxisListType.X)
            # is this tile's max a new running max? (before m_run update)
            cmp = small.tile([B, 1], F32, tag="cmp")
            nc.vector.tensor_tensor(cmp[:], mt[:], m_run[:], mybir.AluOpType.is_gt)
            # within-tile argmax: (logits >= mt) * iota, then row max
            iv = pwork.tile([B, TILE], F32, tag="iv")
            nc.vector.scalar_tensor_tensor(
                iv[:], logits[:], mt[:], iota_s[:],
                mybir.AluOpType.is_ge, mybir.AluOpType.mult,
            )
            it = small.tile([B, 1], F32, tag="it")
            nc.vector.reduce_max(it[:], iv[:], axis=mybir.AxisListType.X)
            # i_run += cmp * (it + i*TILE - i_run)
            dd = small.tile([B, 1], F32, tag="dd")
            nc.vector.tensor_scalar_add(dd[:], it[:], float(i * TILE))
            nc.vector.tensor_sub(dd[:], dd[:], i_run[:])
            nc.vector.scalar_tensor_tensor(
                i_run[:], dd[:], cmp[:], i_run[:],
                mybir.AluOpType.mult, mybir.AluOpType.add,
            )

            mnew = small.tile([B, 1], F32, tag="mnew")
            nc.vector.tensor_max(mnew[:], mt[:], m_run[:])
            dm = small.tile([B, 1], F32, tag="dm")
            nc.vector.tensor_sub(dm[:], m_run[:], mnew[:])
            fsc = small.tile([B, 1], F32, tag="fsc")
            nc.scalar.activation(fsc[:], dm[:], mybir.ActivationFunctionType.Exp)
            nc.vector.tensor_copy(m_run[:], mnew[:])
            negm = small.tile([B, 1], F32, tag="negm")
            nc.vector.tensor_scalar_mul(negm[:], mnew[:], -1.0)

            p = pwork.tile([B, TILE], F32, tag="p")
            rowsum = small.tile([B, 1], F32, tag="rowsum")
            nc.scalar.activation(
                p[:], logits[:], mybir.ActivationFunctionType.Exp,
                bias=negm[:], scale=1.0, accum_out=rowsum[:],
            )
            nc.vector.scalar_tensor_tensor(
                s_run[:], s_run[:], fsc[:], rowsum[:],
                mybir.AluOpType.mult, mybir.AluOpType.add,
            )

            pTp = ps_pt.tile([128, 128], F32, tag="pT")
            for cb in range(4):
                nc.tensor.transpose(
                    pTp[:, cb * 32 : (cb + 1) * 32],
                    p[:, cb * 128 : (cb + 1) * 128],
                    ident_s[:],
                )
            pT8 = pwork.tile([128, 4, 32], FP8, tag="pT8")
            nc.vector.tensor_copy(pT8[:], pTp[:])

            # W += p.T @ train via fp8 DoubleRow (2 n-chunks per matmul)
            wp = ps_w.tile([B, D], F32, tag="wp")
            for g in range(2):
                for jb in range(D // 512):
                    sl = slice(jb * 512, (jb + 1) * 512)
                    nc.tensor.matmul(
                        wp[:, sl],
                        pT8[:, 2 * g : 2 * g + 2, :],
                        b8_t[:, 2 * g : 2 * g + 2, sl],
                        start=(g == 0),
                        stop=(g == 1),
                        perf_mode=DR,
                    )
            nc.vector.scalar_tensor_tensor(
                W_acc[:], W_acc[:], fsc[:], wp[:],
                mybir.AluOpType.mult, mybir.AluOpType.add,
            )

        # repeat>1 is a timing mode: loop the whole pass on-device so the
        # NEFF size stays constant and per-pass time can be measured by slope
        if repeat > 1:
            with tc.For_i(0, repeat):
                for i in range(n_tiles):
                    emit_tile(i)
        else:
            for i in range(n_tiles):
                emit_tile(i)

        nc.sync.dma_start(w_out, W_acc[:])
        nc.sync.dma_start(m_out, m_run[:])
        nc.sync.dma_start(s_out, s_run[:])
        nc.sync.dma_start(i_out, i_run[:])

    nc.compile()
    return nc


_NC_CACHE = {}


def _get_nc(n_tiles=N_TILES):
    if n_tiles not in _NC_CACHE:
        _NC_CACHE[n_tiles] = build_nc(n_tiles)
    return _NC_CACHE[n_tiles]


LAST_RESULT = None  # BassKernelResults of the most recent run (for test harness)
LAST_IN_MAPS = None  # per-core input dicts of the most recent run


def kernel(x, train_data, alphas_cumprod, t):
    x = np.asarray(x)
    train_data = np.asarray(train_data)
    alphas_cumprod = np.asarray(alphas_cumprod)
    t_idx = int(np.asarray(t))

    ab = float(alphas_cumprod[t_idx])
    s_ab = np.sqrt(ab)
    one_minus = 1.0 - ab
    coefA = s_ab / one_minus            # logits = coefA * (x . t) - coefB * t_sq
    coefB = ab / (2.0 * one_minus)
    inv = 1.0 / np.sqrt(one_minus)

    xf = x.reshape(B, D).astype(np.float64)
    xs = coefA * xf                      # fold coefA into the query side

    # x-side stationary operand (shared across cores): e4m3 hi + e4m3 lo(x16);
    # second set pre-scaled (1/16, 1/256) for the lo pass
    x8h = xs.astype(NP_FP8)
    x8l = ((xs - x8h.astype(np.float64)) * LO_SCALE).astype(NP_FP8)
    x8h_s = (x8h.astype(np.float64) / 16.0).astype(NP_FP8)
    x8l_s = (x8l.astype(np.float64) / 256.0).astype(NP_FP8)
    xw8 = np.zeros((2, KP, 128, 2, 64), NP_FP8)
    for jp in range(KP):
        for r in range(2):
            sl = slice((2 * jp + r) * 128, (2 * jp + r + 1) * 128)
            xw8[0, jp, :, r, 0:B] = x8h[:, sl].T
            xw8[0, jp, :, r, B:64] = x8l[:, sl].T
            xw8[1, jp, :, r, 0:B] = x8h_s[:, sl].T
            xw8[1, jp, :, r, B:64] = x8l_s[:, sl].T
    xwb = np.zeros((2, 64), np.float16)
    xwb[0, 0:B] = 1.0
    xwb[1, 0:B] = 1.0
    ident = np.eye(32, dtype=np.float32)
    iota = np.broadcast_to(
        np.arange(TILE, dtype=np.float32)[None, :], (B, TILE)
    ).copy()

    tf = train_data.reshape(N, D)
    in_maps = []
    for c in range(N_CORES):
        shard = tf[c * N_SHARD : (c + 1) * N_SHARD].astype(np.float32)
        t_pad = np.zeros((N_PAD, D), np.float32)
        t_pad[:N_SHARD] = shard

        td = t_pad.astype(np.float64)
        t_sq = np.einsum("nd,nd->n", td, td)
        bias = -coefB * (t_sq - float(D))
        bias[N_SHARD:] = PAD_BIAS

        At = td.T                                    # [D, N_PAD]
        A_h8 = At.astype(NP_FP8)
        A_l8 = ((At - A_h8.astype(np.float64)) * LO_SCALE).astype(NP_FP8)
        # [tile, p, pair, 2, n] partition-major layout
        a8h_c = np.ascontiguousarray(
            A_h8.reshape(KP, 2, 128, N_TILES, TILE).transpose(3, 2, 0, 1, 4)
        )
        a8l_c = np.ascontiguousarray(
            A_l8.reshape(KP, 2, 128, N_TILES, TILE).transpose(3, 2, 0, 1, 4)
        )

        bias_hi = bias.astype(np.float16)
        bias_lo = (bias - bias_hi.astype(np.float64)).astype(np.float16)
        a_b = (
            np.stack([bias_hi, bias_lo])             # [2, N_PAD]
            .reshape(2, N_TILES, TILE)
            .astype(np.float16)
        )

        b8_c = np.ascontiguousarray(
            t_pad.astype(NP_FP8).reshape(N_TILES, 4, 128, D).transpose(0, 2, 1, 3)
        )

        in_maps.append(
            dict(
                a8h=a8h_c, a8l=a8l_c, a_b=a_b, b8=b8_c,
                xw8=xw8, xwb=xwb, ident=ident, iota=iota,
            )
        )

    nc = _get_nc()
    res = bass_utils.run_bass_kernel_spmd(nc, in_maps, core_ids=list(range(N_CORES)))
    global LAST_RESULT, LAST_IN_MAPS
    LAST_RESULT = res
    LAST_IN_MAPS = in_maps

    Wc = np.stack([r["w_out"] for r in res.results]).astype(np.float64)  # [8,B,D]
    mc = np.stack([r["m_out"][:, 0] for r in res.results]).astype(np.float64)
    sc = np.stack([r["s_out"][:, 0] for r in res.results]).astype(np.float64)
    ic = np.stack([r["i_out"][:, 0] for r in res.results])               # [8,B]

    # repair: each core's argmax row entered W with coefficient exactly 1.0
    # (exp(0) at the tile that set the final max, never rescaled after);
    # swap its fp8-quantized value for the exact f32 row.
    tf64 = tf.astype(np.float64)
    for c in range(N_CORES):
        for q in range(B):
            il = int(round(float(ic[c, q])))
            pos = c * N_SHARD + il
            row32 = tf[pos].astype(np.float32)
            row8 = row32.astype(NP_FP8).astype(np.float64)
            Wc[c, q] += tf64[pos] - row8

    M = mc.max(0)                                    # [B]
    fac = np.exp(mc - M[None, :])                    # [8, B]
    W_tot = np.einsum("cb,cbd->bd", fac, Wc)
    s_tot = (fac * sc).sum(0)                        # [B]
    weighted = W_tot / s_tot[:, None]                # [B, D]

    out = inv * xf - (s_ab * inv) * weighted
    return out.reshape(x.shape).astype(np.float32)



# revision 5
# speedup vs baseline: 2.2354x; 2.2354x over previous
"""Trainium2 Bass kernel for nn_DiffusionStar (retrieval_knn).

Computes eps_star = (x - sqrt(ab) * weighted_x) / sqrt(1 - ab) where
weighted_x is the softmax-weighted average of the train set under the
Gaussian kernel exp(-||x - sqrt(ab) x0||^2 / (2 (1 - ab))).

Two-stage retrieval design (the softmax is ~1-hot: at most ~9 rows per
query fall within 34 logits of the max, and mass below max-34 is <1e-15):

 - Device (8 cores, train sharded along N): stream a d-major fp8(e4m3)
   copy of the shard once -- 1 byte/element, the HBM-roofline cost --
   and emit coarse cross-product rows: c[0:32] = x8_hi . t8,
   c[32:64] = x8_lo . t8 (x split hi + 16*residual so the x side adds
   no error; the only noise is the train fp8 quantization,
   empirically |err| <= 36 logits).
 - Host: coarse_logit = (hi + lo/16) - coefB*(t_sq - D); every row
   within DELTA=150 of any query's coarse max (~250 rows total,
   worst-case margin needs only 82) is rescored exactly in f64 and the
   softmax + weighted average is computed over those candidates only.
"""

import contextlib

import ml_dtypes
import numpy as np

from concourse import bacc, bass, mybir, tile
from concourse import bass_utils

FP16 = mybir.dt.float16
FP8 = mybir.dt.float8e4
F32 = mybir.dt.float32
NP_FP8 = ml_dtypes.float8_e4m3

B = 32          # queries
D = 3072        # feature dim (c*h*w)
N = 100000      # train points
N_CORES = 8
N_SHARD = N // N_CORES          # 12500
TILE = 512
N_TILES = (N_SHARD + TILE - 1) // TILE   # 25
N_PAD = N_TILES * TILE                   # 12800
KC = D // 128                            # 24 contraction chunks
KP = KC // 2                             # 12 DoubleRow chunk pairs
LO_SCALE = 16.0                          # fp8 lo-residual scale (x side)
DELTA = 150.0                            # host candidate threshold
DR = mybir.MatmulPerfMode.DoubleRow


def build_nc(n_tiles=N_TILES, repeat=1):
    nc = bacc.Bacc("TRN2", target_bir_lowering=False, debug=False, num_devices=1)

    # d-major fp8 operand: [tile, p, pair, 2, n] so each SBUF partition's
    # bytes are one contiguous HBM run
    a8h = nc.dram_tensor(
        "a8h", [n_tiles, 128, KP, 2, TILE], FP8, kind="ExternalInput"
    ).ap()
    # x-side stationary: hi | lo(x16) packed as 64 output columns
    xw8 = nc.dram_tensor("xw8", [128, KP, 2, 64], FP8, kind="ExternalInput").ap()

    lg_out = nc.dram_tensor("lg_out", [64, n_tiles, TILE], FP16,
                            kind="ExternalOutput").ap()

    with tile.TileContext(nc) as tc, contextlib.ExitStack() as st:
        const = st.enter_context(tc.tile_pool(name="const", bufs=1))
        apool = st.enter_context(tc.tile_pool(name="apool", bufs=3))
        ps_c1 = st.enter_context(tc.tile_pool(name="ps_c1", bufs=2, space="PSUM"))

        xw8_s = const.tile([128, KP, 2, 64], FP8)
        nc.sync.dma_start(xw8_s[:], xw8)
        logbuf = const.tile([64, n_tiles, TILE], FP16)

        def emit_tile(i):
            a8h_t = apool.tile([128, KP, 2, TILE], FP8, tag="a8h")
            # split across the two HW DGE queues (SP + Act)
            nc.sync.dma_start(a8h_t[:, 0 : KP // 2], a8h[i, :, 0 : KP // 2])
            nc.scalar.dma_start(a8h_t[:, KP // 2 : KP], a8h[i, :, KP // 2 : KP])

            c1 = ps_c1.tile([64, TILE], F32, tag="c1")
            for j in range(KP):
                nc.tensor.matmul(
                    c1[:], xw8_s[:, j], a8h_t[:, j],
                    start=(j == 0), stop=(j == KP - 1), perf_mode=DR,
                )
            nc.scalar.copy(logbuf[:, i], c1[:])

        def emit_pass():
            for i in range(n_tiles):
                emit_tile(i)
            nc.sync.dma_start(lg_out, logbuf[:])

        # repeat>1 is a timing mode: loop the whole pass on-device so the
        # NEFF size stays constant and per-pass time can be measured by slope
        if repeat > 1:
            with tc.For_i(0, repeat):
                emit_pass()
        else:
            emit_pass()

    nc.compile()
    return nc


_NC_CACHE = {}


def _get_nc(n_tiles=N_TILES):
    if n_tiles not in _NC_CACHE:
        _NC_CACHE[n_tiles] = build_nc(n_tiles)
    return _NC_CACHE[n_tiles]


LAST_RESULT = None  # BassKernelResults of the most recent run (for test harness)
LAST_IN_MAPS = None  # per-core input dicts of the most recent run


def kernel(x, train_data, alphas_cumprod, t):
    x = np.asarray(x)
    train_data = np.asarray(train_data)
    alphas_cumprod = np.asarray(alphas_cumprod)
    t_idx = int(np.asarray(t))

    ab = float(alphas_cumprod[t_idx])
    s_ab = np.sqrt(ab)
    one_minus = 1.0 - ab
    coefA = s_ab / one_minus            # logits = coefA * (x . t) - coefB * t_sq
    coefB = ab / (2.0 * one_minus)
    inv = 1.0 / np.sqrt(one_minus)

    xf = x.reshape(B, D).astype(np.float64)
    xs = coefA * xf                      # fold coefA into the query side

    # x-side stationary operand (shared across cores): e4m3 hi + e4m3 lo(x16)
    x8h = xs.astype(NP_FP8)
    x8l = ((xs - x8h.astype(np.float64)) * LO_SCALE).astype(NP_FP8)
    xw8 = np.zeros((KP, 128, 2, 64), NP_FP8)
    for jp in range(KP):
        for r in range(2):
            sl = slice((2 * jp + r) * 128, (2 * jp + r + 1) * 128)
            xw8[jp, :, r, 0:B] = x8h[:, sl].T
            xw8[jp, :, r, B:64] = x8l[:, sl].T
    xw8_dev = np.ascontiguousarray(xw8.transpose(1, 0, 2, 3))  # [128, KP, 2, 64]

    tf = train_data.reshape(N, D)
    in_maps = []
    for c in range(N_CORES):
        shard = tf[c * N_SHARD : (c + 1) * N_SHARD].astype(np.float32)
        t_pad = np.zeros((N_PAD, D), np.float32)
        t_pad[:N_SHARD] = shard
        A_h8 = t_pad.T.astype(NP_FP8)                # [D, N_PAD]
        # [tile, p, pair, 2, n] partition-major layout
        a8h_c = np.ascontiguousarray(
            A_h8.reshape(KP, 2, 128, N_TILES, TILE).transpose(3, 2, 0, 1, 4)
        )
        in_maps.append(dict(a8h=a8h_c, xw8=xw8_dev))

    nc = _get_nc()
    res = bass_utils.run_bass_kernel_spmd(nc, in_maps, core_ids=list(range(N_CORES)))
    global LAST_RESULT, LAST_IN_MAPS
    LAST_RESULT = res
    LAST_IN_MAPS = in_maps

    # coarse logits from device cross rows + host bias
    lg = np.stack(
        [r["lg_out"].reshape(64, N_PAD) for r in res.results]
    ).astype(np.float64)                                                  # [8,64,N_PAD]
    coarse_cross = lg[:, 0:B, :N_SHARD] + lg[:, B:64, :N_SHARD] / LO_SCALE
    coarse_cross = np.concatenate(list(coarse_cross), axis=1)             # [B, N]

    tf64 = tf.astype(np.float64)
    t_sq = np.einsum("nd,nd->n", tf64, tf64)
    bias = -coefB * (t_sq - float(D))
    Lc = coarse_cross + bias[None, :]

    mh = Lc.max(axis=1)
    cand = (Lc >= mh[:, None] - DELTA).any(axis=0)
    idx = np.nonzero(cand)[0]

    # exact rescore of candidates in f64
    sub = tf64[idx]                                  # [C, D]
    L_e = coefA * (xf @ sub.T) + bias[idx][None, :]  # [B, C]
    m_e = L_e.max(axis=1)
    P = np.exp(L_e - m_e[:, None])
    s_tot = P.sum(axis=1)
    weighted = (P @ sub) / s_tot[:, None]            # [B, D]

    out = inv * xf - (s_ab * inv) * weighted
    return out.reshape(x.shape).astype(np.float32)


# revision 9
# speedup vs baseline: 3.0054x; 1.3445x over previous
"""Trainium2 Bass kernel for nn_DiffusionStar (retrieval_knn).

Computes eps_star = (x - sqrt(ab) * weighted_x) / sqrt(1 - ab) where
weighted_x is the softmax-weighted average of the train set under the
Gaussian kernel exp(-||x - sqrt(ab) x0||^2 / (2 (1 - ab))).

Two-stage retrieval design (the softmax is ~1-hot: at most ~9 rows per
query fall within 34 logits of the max, and mass below max-34 is <1e-15):

 - Device (8 cores, train sharded along N): stream a d-major fp8(e4m3)
   copy of the shard once -- 1 byte/element, the HBM-roofline cost --
   and emit coarse cross-product rows: c[0:32] = x8_hi . t8,
   c[32:64] = x8_lo . t8 (x split hi + 16*residual so the x side adds
   no error; the only noise is the train fp8 quantization,
   empirically |err| <= 36 logits).
 - Host: coarse_logit = (hi + lo/16) - coefB*(t_sq - D); every row
   within DELTA=150 of any query's coarse max (~250 rows total,
   worst-case margin needs only 82) is rescored exactly in f64 and the
   softmax + weighted average is computed over those candidates only.
"""

import contextlib

import ml_dtypes
import numpy as np

from concourse import bacc, bass, mybir, tile
from concourse import bass_utils

FP16 = mybir.dt.float16
FP8 = mybir.dt.float8e4
F32 = mybir.dt.float32
NP_FP8 = ml_dtypes.float8_e4m3

B = 32          # queries
D = 3072        # feature dim (c*h*w)
N = 100000      # train points
N_CORES = 8
N_SHARD = N // N_CORES          # 12500
TILE = 512
N_TILES = (N_SHARD + TILE - 1) // TILE   # 25
N_PAD = N_TILES * TILE                   # 12800
KC = D // 128                            # 24 contraction chunks
KP = KC // 2                             # 12 DoubleRow chunk pairs
LO_SCALE = 16.0                          # fp8 lo-residual scale (x side)
DELTA = 150.0                            # host candidate threshold
DR = mybir.MatmulPerfMode.DoubleRow


def build_nc(n_tiles=N_TILES, repeat=1, skip_compute=False, skip_dma=False):
    nc = bacc.Bacc("TRN2", target_bir_lowering=False, debug=False, num_devices=1)

    # d-major fp8 operand: [tile, p, pair, 2, n] so each SBUF partition's
    # bytes are one contiguous HBM run
    a8h = nc.dram_tensor(
        "a8h", [n_tiles, 128, KP, 2, TILE], FP8, kind="ExternalInput"
    ).ap()
    # x-side stationary: hi | lo(x16) packed as 64 output columns
    xw8 = nc.dram_tensor("xw8", [128, KP, 2, 64], FP8, kind="ExternalInput").ap()

    lg_out = nc.dram_tensor("lg_out", [64, n_tiles, TILE], FP16,
                            kind="ExternalOutput").ap()

    with tile.TileContext(nc) as tc, contextlib.ExitStack() as st:
        const = st.enter_context(tc.tile_pool(name="const", bufs=1))
        apool = st.enter_context(tc.tile_pool(name="apool", bufs=4))
        ps_c1 = st.enter_context(tc.tile_pool(name="ps_c1", bufs=2, space="PSUM"))

        xw8_s = const.tile([128, KP, 2, 64], FP8)
        nc.sync.dma_start(xw8_s[:], xw8)
        logbuf = const.tile([64, n_tiles, TILE], FP16)

        def emit_tile(i):
            a8h_t = apool.tile([128, KP, 2, TILE], FP8, tag="a8h")
            # one big transfer per tile, alternating between the two HW DGE
            # queues (SP/Act); keep those engines free of non-DMA work so
            # their strict FIFOs never stall the DMA pipeline
            eng = nc.sync if (i % 2 == 0) else nc.scalar
            if not skip_dma:
                eng.dma_start(a8h_t[:], a8h[i])
            if skip_compute:
                dmy = apool.tile([128, 1], F32, tag="dmy")
                nc.vector.reduce_max(dmy[:], a8h_t[:, 0, 0, 0:8],
                                     axis=mybir.AxisListType.X)
                return

            c1 = ps_c1.tile([64, TILE], F32, tag="c1")
            for j in range(KP):
                nc.tensor.matmul(
                    c1[:], xw8_s[:, j], a8h_t[:, j],
                    start=(j == 0), stop=(j == KP - 1), perf_mode=DR,
                )
            nc.vector.tensor_copy(logbuf[:, i], c1[:])

        def emit_pass():
            for i in range(n_tiles):
                emit_tile(i)
            nc.sync.dma_start(lg_out, logbuf[:])

        # repeat>1 is a timing mode: loop the whole pass on-device so the
        # NEFF size stays constant and per-pass time can be measured by slope
        if repeat > 1:
            with tc.For_i(0, repeat):
                emit_pass()
        else:
            emit_pass()

    nc.compile()
    return nc


_NC_CACHE = {}


def _get_nc(n_tiles=N_TILES):
    if n_tiles not in _NC_CACHE:
        _NC_CACHE[n_tiles] = build_nc(n_tiles)
    return _NC_CACHE[n_tiles]


LAST_RESULT = None  # BassKernelResults of the most recent run (for test harness)
LAST_IN_MAPS = None  # per-core input dicts of the most recent run


def kernel(x, train_data, alphas_cumprod, t):
    x = np.asarray(x)
    train_data = np.asarray(train_data)
    alphas_cumprod = np.asarray(alphas_cumprod)
    t_idx = int(np.asarray(t))

    ab = float(alphas_cumprod[t_idx])
    s_ab = np.sqrt(ab)
    one_minus = 1.0 - ab
    coefA = s_ab / one_minus            # logits = coefA * (x . t) - coefB * t_sq
    coefB = ab / (2.0 * one_minus)
    inv = 1.0 / np.sqrt(one_minus)

    xf = x.reshape(B, D).astype(np.float64)
    xs = coefA * xf                      # fold coefA into the query side

    # x-side stationary operand (shared across cores): e4m3 hi + e4m3 lo(x16)
    x8h = xs.astype(NP_FP8)
    x8l = ((xs - x8h.astype(np.float64)) * LO_SCALE).astype(NP_FP8)
    xw8 = np.zeros((KP, 128, 2, 64), NP_FP8)
    for jp in range(KP):
        for r in range(2):
            sl = slice((2 * jp + r) * 128, (2 * jp + r + 1) * 128)
            xw8[jp, :, r, 0:B] = x8h[:, sl].T
            xw8[jp, :, r, B:64] = x8l[:, sl].T
    xw8_dev = np.ascontiguousarray(xw8.transpose(1, 0, 2, 3))  # [128, KP, 2, 64]

    tf = train_data.reshape(N, D)
    in_maps = []
    for c in range(N_CORES):
        shard = tf[c * N_SHARD : (c + 1) * N_SHARD].astype(np.float32)
        t_pad = np.zeros((N_PAD, D), np.float32)
        t_pad[:N_SHARD] = shard
        A_h8 = t_pad.T.astype(NP_FP8)                # [D, N_PAD]
        # [tile, p, pair, 2, n] partition-major layout
        a8h_c = np.ascontiguousarray(
            A_h8.reshape(KP, 2, 128, N_TILES, TILE).transpose(3, 2, 0, 1, 4)
        )
        in_maps.append(dict(a8h=a8h_c, xw8=xw8_dev))

    nc = _get_nc()
    res = bass_utils.run_bass_kernel_spmd(nc, in_maps, core_ids=list(range(N_CORES)))
    global LAST_RESULT, LAST_IN_MAPS
    LAST_RESULT = res
    LAST_IN_MAPS = in_maps

    # coarse logits from device cross rows + host bias
    lg = np.stack(
        [r["lg_out"].reshape(64, N_PAD) for r in res.results]
    ).astype(np.float64)                                                  # [8,64,N_PAD]
    coarse_cross = lg[:, 0:B, :N_SHARD] + lg[:, B:64, :N_SHARD] / LO_SCALE
    coarse_cross = np.concatenate(list(coarse_cross), axis=1)             # [B, N]

    tf64 = tf.astype(np.float64)
    t_sq = np.einsum("nd,nd->n", tf64, tf64)
    bias = -coefB * (t_sq - float(D))
    Lc = coarse_cross + bias[None, :]

    mh = Lc.max(axis=1)
    cand = (Lc >= mh[:, None] - DELTA).any(axis=0)
    idx = np.nonzero(cand)[0]

    # exact rescore of candidates in f64
    sub = tf64[idx]                                  # [C, D]
    L_e = coefA * (xf @ sub.T) + bias[idx][None, :]  # [B, C]
    m_e = L_e.max(axis=1)
    P = np.exp(L_e - m_e[:, None])
    s_tot = P.sum(axis=1)
    weighted = (P @ sub) / s_tot[:, None]            # [B, D]

    out = inv * xf - (s_ab * inv) * weighted
    return out.reshape(x.shape).astype(np.float32)
